# revision 1
# baseline (speedup 1.0000x reference)
"""CPC unsupervised criterion loss on 8 Trainium2 NeuronCores.

Strategy (data-parallel over batch B=8, one batch row per core):
  - The irregular 121 MB negative-sample gather is replaced by a dense
    score matrix: for each (k, w) we compute scores against ALL B*S=1024
    encoder rows via PE matmuls. Sampled-negative multiplicities cnt[w,j]
    are built on the host from the index tensors, so
      * sum_n exp(negScore_n) = sum_j cnt[w,j]*exp(score[w,j]),
      * max_n negScore_n = max over {j : cnt[w,j]>0} of score[w,j].
  - All score matmuls run in bf16 (4x PE rate vs fp32). Exact-tie
    behaviour of the reference argmax (positive drawn as its own
    negative, ~12% of positions) is still preserved bit-exactly because
    the positive and the duplicate negative are read from the SAME PSUM
    value. bf16 noise shifts per-position losses by ~1e-3 which averages
    to <4e-5 in the final mean; near-margin accuracy bits
    (|pos - maxneg| < 0.02, ~100 per run) are re-resolved exactly on the
    host in float64.
  - The positive score is extracted bit-exactly from the PSUM score row
    via a one-hot multiply + sum (scalar_tensor_tensor with accum_out).
  - The -60000 sampling mask and the ln(cnt) duplicate weights are added
    into PSUM with fp16 identity matmuls (fp16 keeps ln(cnt) to ~1e-4),
    so negsum comes for free from the ScalarE Exp pass's accum_out and
    the Vector engine only runs one reduce_max and one small extraction
    per k.
  - Constants are packed into blob tensors (fp32 / bf16 / fp16) so the
    startup needs only a handful of DMA descriptor pushes; the bf16
    wpredT streams in three progressively-available chunks.
  - Per-core outputs are tiny (116 x 36); accuracy comparison and the
    final mean over (B, W) happen on the host.
"""

import numpy as np

B, S, K, D, NNEG = 8, 128, 12, 256, 128
W = S - K          # 116
J = B * S          # 1024
NCORES = 8
MASK_NEG = -60000.0      # fp16-representable "-inf" for unsampled columns
MARGIN_TAU = 0.02        # host re-check window around pos ~ maxneg

# bf16 blob columns: flatT | cT
BB_FT, BB_CT = 0, 2 * J
BB_END = BB_CT + 2 * W                               # 2280
# fp16 blob columns: negbias | delta(=ln cnt) | ident
H_NB, H_DL, H_ID = 0, J, 2 * J
H_END = H_ID + W                                     # 2164

_CACHE = {}


def _build_program():
    from concourse import bacc, mybir
    import concourse.tile as tile

    f32 = mybir.dt.float32
    bf16 = mybir.dt.bfloat16
    f16 = mybir.dt.float16
    Alu = mybir.AluOpType
    Act = mybir.ActivationFunctionType

    nc = bacc.Bacc(
        "TRN2", target_bir_lowering=False, debug=False, num_devices=NCORES
    )

    fb_d = nc.dram_tensor("fblob", [128, S + K], f32, kind="ExternalInput")
    bb_d = nc.dram_tensor("bblob", [128, BB_END], bf16, kind="ExternalInput")
    hb_d = nc.dram_tensor("hblob", [128, H_END], f16, kind="ExternalInput")
    wp_d = nc.dram_tensor("wpredT", [128, K * 2 * D], bf16, kind="ExternalInput")
    out_d = nc.dram_tensor("out", [W, 3 * K], f32, kind="ExternalOutput")

    with tile.TileContext(nc) as tc:
        with (
            tc.tile_pool(name="consts", bufs=1) as consts,
            tc.tile_pool(name="lcpool", bufs=4) as lcpool,
            tc.tile_pool(name="scr", bufs=4) as scr,
            tc.tile_pool(name="outs", bufs=1) as outs,
            tc.tile_pool(name="pslc", bufs=2, space="PSUM") as pslc,
            tc.tile_pool(name="pssc", bufs=3, space="PSUM") as pssc,
        ):
            bb = consts.tile([128, BB_END], bf16)
            wpall = consts.tile([128, K * 2 * D], bf16)
            fb = consts.tile([128, S + K], f32)
            hb = consts.tile([128, H_END], f16)
            nc.sync.dma_start(bb[:, BB_CT:BB_END], bb_d[:, BB_CT:BB_END])
            nc.sync.dma_start(wpall[:, 0:512], wp_d[:, 0:512])
            nc.sync.dma_start(bb[:, 0:BB_CT], bb_d[:, 0:BB_CT])
            nc.sync.dma_start(fb[:], fb_d[:])
            nc.sync.dma_start(hb[:, H_ID:H_END], hb_d[:, H_ID:H_END])
            nc.sync.dma_start(hb[:, 0:H_ID], hb_d[:, 0:H_ID])
            nc.sync.dma_start(wpall[:, 512:2048], wp_d[:, 512:2048])
            nc.sync.dma_start(wpall[:, 2048:K * 512], wp_d[:, 2048:K * 512])

            fT_v = bb[:, BB_FT:BB_FT + 2 * J]
            cT_v = bb[:, BB_CT:BB_CT + 2 * W]
            ohb = fb[0:W, 0:S + K]
            nb_v = hb[0:W, H_NB:H_NB + J]
            dl_v = hb[0:W, H_DL:H_DL + J]
            id_v = hb[0:W, H_ID:H_ID + W]

            posS = outs.tile([W, K], f32)
            maxneg = outs.tile([W, K], f32)
            negsum = outs.tile([W, K], f32)

            for k in range(K):
                wk = wpall[:, k * 2 * D:(k + 1) * 2 * D]

                # locC_T[k]: (e', ec*W + w) = sum_d WpredT[d, e] * cT[d, w]
                lcT_ps = pslc.tile([128, 2 * W], f32, tag="lcT")
                for ec in range(2):
                    for dc in range(2):
                        nc.tensor.matmul(
                            lcT_ps[:, ec * W:(ec + 1) * W],
                            lhsT=wk[:, dc * D + ec * 128: dc * D + (ec + 1) * 128],
                            rhs=cT_v[:, dc * W:(dc + 1) * W],
                            start=(dc == 0),
                            stop=(dc == 1),
                        )
                lcT_bf = lcpool.tile([128, 2 * W], bf16, tag="lcT_bf")
                nc.scalar.copy(lcT_bf[:], lcT_ps[:])

                # scores (bf16): (w, j) = sum_e locC_T[e, w] * flatT[e, j]
                sc_ps = pssc.tile([W, J], f32, tag="sc")
                for jc in range(2):
                    for ec in range(2):
                        nc.tensor.matmul(
                            sc_ps[:, jc * 512:(jc + 1) * 512],
                            lhsT=lcT_bf[:, ec * W:(ec + 1) * W],
                            rhs=fT_v[:, ec * J + jc * 512: ec * J + (jc + 1) * 512],
                            start=(ec == 0),
                            stop=False,
                            skip_group_check=True,
                        )

                # positive score: exact one-hot extraction at column k+1+w
                # (ohbase[w, c] = 1 iff c == w+13; slicing at 12-k aligns
                # the hot column to j' = w+k+1)
                scrP = scr.tile([W, S], f32, tag="scrP")
                nc.vector.scalar_tensor_tensor(
                    out=scrP[:],
                    in0=sc_ps[:, 0:S],
                    scalar=1.0,
                    in1=ohb[:, K - k:K - k + S],
                    op0=Alu.mult,
                    op1=Alu.mult,
                    accum_out=posS[:, k:k + 1],
                )

                # scores += negbias (0.0 where sampled, -60000 elsewhere)
                for jc in range(2):
                    nc.tensor.matmul(
                        sc_ps[:, jc * 512:(jc + 1) * 512],
                        lhsT=id_v,
                        rhs=nb_v[:, jc * 512:(jc + 1) * 512],
                        start=False,
                        stop=False,
                        skip_group_check=True,
                    )
                # max over sampled negatives (exact scores where sampled)
                nc.vector.reduce_max(
                    maxneg[:, k:k + 1], sc_ps[:], axis=mybir.AxisListType.X
                )
                # scores += ln(cnt) (0.0 where unsampled)
                for jc in range(2):
                    nc.tensor.matmul(
                        sc_ps[:, jc * 512:(jc + 1) * 512],
                        lhsT=id_v,
                        rhs=dl_v[:, jc * 512:(jc + 1) * 512],
                        start=False,
                        stop=True,
                        skip_group_check=True,
                    )
                # negsum = sum_j cnt * exp(score) via Exp with accumulate
                scrB = scr.tile([W, J], bf16, tag="scrB")
                nc.scalar.activation(
                    out=scrB[:],
                    in_=sc_ps[:],
                    func=Act.Exp,
                    accum_out=negsum[:, k:k + 1],
                )

            nc.sync.dma_start(out_d[:, 0:K], negsum[:])
            nc.sync.dma_start(out_d[:, K:2 * K], posS[:])
            nc.sync.dma_start(out_d[:, 2 * K:3 * K], maxneg[:])

    nc.compile()
    return nc


def _host_prep(cFeature, encodedData, Wpred, batchIdx, seqIdx):
    import ml_dtypes

    bf = ml_dtypes.bfloat16
    cF = np.ascontiguousarray(np.asarray(cFeature, dtype=np.float32))
    eD = np.ascontiguousarray(np.asarray(encodedData, dtype=np.float32))
    Wp = np.ascontiguousarray(np.asarray(Wpred, dtype=np.float32))
    bI = np.asarray(batchIdx).astype(np.int64)
    sI = np.asarray(seqIdx).astype(np.int64)

    flat = eD.reshape(J, D)
    idx = np.arange(NNEG * W * B, dtype=np.int64)
    ext = ((sI + idx % W) % S + bI * S).reshape(B, NNEG, W)

    wt = Wp.transpose(0, 2, 1)  # (K, d, e)
    wp_host = np.concatenate(
        [np.concatenate([wt[k, :128, :], wt[k, 128:, :]], axis=1) for k in range(K)],
        axis=1,
    ).astype(bf)  # (128, K*2D)
    wp_host = np.ascontiguousarray(wp_host)

    fblob = np.zeros((128, S + K), np.float32)
    fblob[np.arange(W), np.arange(W) + K + 1] = 1.0

    rows = np.tile(np.arange(W), NNEG)
    in_maps = []
    cnts_orig = []
    for b in range(B):
        perm = np.r_[b * S:(b + 1) * S, 0:b * S, (b + 1) * S:J]
        inv = np.empty(J, np.int64)
        inv[perm] = np.arange(J)

        fT = flat[perm].T  # (D, J) fp32
        cT = cF[b, :W].T * np.float32(1.0 / 256.0)  # exact power-of-2 scale

        cnt = np.zeros((W, J), np.float32)
        np.add.at(cnt, (rows, inv[ext[b].ravel()]), 1.0)
        cnt_o = np.zeros((W, J), np.float32)
        np.add.at(cnt_o, (rows, ext[b].ravel()), 1.0)
        cnts_orig.append(cnt_o)
        nz = cnt > 0

        bblob = np.zeros((128, BB_END), bf)
        bblob[:, BB_FT:BB_FT + J] = fT[:128].astype(bf)
        bblob[:, BB_FT + J:BB_FT + 2 * J] = fT[128:].astype(bf)
        bblob[:, BB_CT:BB_CT + W] = cT[:128].astype(bf)
        bblob[:, BB_CT + W:BB_CT + 2 * W] = cT[128:].astype(bf)

        hblob = np.zeros((128, H_END), np.float16)
        hblob[:W, H_NB:H_NB + J] = np.where(nz, 0.0, MASK_NEG).astype(np.float16)
        dl = np.zeros((W, J), np.float32)
        dl[nz] = np.log(cnt[nz])
        hblob[:W, H_DL:H_DL + J] = dl.astype(np.float16)
        hblob[:W, H_ID:H_ID + W] = np.eye(W, dtype=np.float16)

        in_maps.append({
            "fblob": fblob,
            "bblob": np.ascontiguousarray(bblob),
            "hblob": np.ascontiguousarray(hblob),
            "wpredT": wp_host,
        })
    return in_maps, cnts_orig, flat, cF, Wp


def _host_fix_acc(acc01, margin, cnts_orig, flat, cF, Wp):
    """Re-resolve near-margin accuracy bits exactly in float64."""
    flat64 = flat.astype(np.float64)
    for b in range(B):
        flag = np.abs(margin[b]) < MARGIN_TAU    # (W, K)
        for w, k in zip(*np.nonzero(flag)):
            lc = (cF[b, w].astype(np.float64) / 256.0) @ Wp[k].astype(np.float64).T
            sc = flat64 @ lc                     # (J,)
            mn = sc[cnts_orig[b][w] > 0].max()
            p = sc[b * S + k + 1 + w]
            acc01[b, w, k] = 1.0 if p >= mn else 0.0
    return acc01


def kernel(cFeature, encodedData, Wpred, batchIdx, seqIdx, _trace=False):
    from concourse.bass_utils import run_bass_kernel_spmd

    in_maps, cnts_orig, flat, cF, Wp = _host_prep(
        cFeature, encodedData, Wpred, batchIdx, seqIdx
    )

    if "nc" not in _CACHE:
        _CACHE["nc"] = _build_program()
    nc = _CACHE["nc"]

    kw = {}
    if _trace:
        kw = {"trace": True}
    res = run_bass_kernel_spmd(nc, in_maps, core_ids=list(range(NCORES)), **kw)
    _CACHE["last_results"] = res

    outs = np.stack([res.results[b]["out"] for b in range(B)])  # (B, W, 3K)
    negsum = outs[:, :, :K].astype(np.float64)
    posS = outs[:, :, K:2 * K]
    maxneg = outs[:, :, 2 * K:3 * K]
    p64 = posS.astype(np.float64)
    lossc = np.log(negsum + np.exp(p64)) - p64

    margin = (posS - maxneg).astype(np.float64)
    acc01 = (margin >= 0).astype(np.float32)
    acc01 = _host_fix_acc(acc01, margin, cnts_orig, flat, cF, Wp)

    losses = lossc.sum(axis=(0, 1), dtype=np.float64) / (B * W)
    accs = acc01.sum(axis=(0, 1), dtype=np.float64) / (B * W)
    return (
        losses.astype(np.float32)[None, :],
        accs.astype(np.float32)[None, :],
    )



# revision 3
# speedup vs baseline: 1.1361x; 1.1361x over previous
"""CPC unsupervised criterion loss on 8 Trainium2 NeuronCores.

Strategy (data-parallel over batch B=8, one batch row per core):
  - The irregular 121 MB negative-sample gather is replaced by a dense
    score matrix: for each (k, w) we compute scores against ALL B*S=1024
    encoder rows via PE matmuls. Sampled-negative multiplicities cnt[w,j]
    are built on the host from the index tensors, so
      sum_n exp(negScore_n) = sum_j cnt[w,j]*exp(score[w,j]).
  - Device pipeline per k (software-pipelined across k):
      PE : locC = WpredT @ cT (4 mm), scores = locC.T @ flatT (4 mm),
           scores += nbdl via fp16 identity mm (2 mm), where
           nbdl = ln(cnt) at sampled columns, -60000 elsewhere.
      ACT: E = Exp(scores + nbdl) -> bf16; accum_out gives
           negsum = sum_j cnt*exp(score) for free. A tiny batched
           Exp(-posS) produces the per-row threshold epn.
      DVE: lcT PSUM->SBUF bf16 copy; exact positive extraction from the
           score PSUM via one-hot scalar_tensor_tensor (accum_out);
           violation count = sum_j [E*epn > cnt] via one
           scalar_tensor_tensor with is_gt (the cnt multiplicity
           cancels: cnt*e^s*e^-pos > cnt  <=>  s > pos).
  - Accuracy bit is count == 0 (argmax ties go to the positive, matching
    jnp.argmax-first semantics). Rows with count <= 1 sit near the
    decision boundary under bf16 noise and are re-resolved exactly on
    the host in float64 (~200 rows).
  - Per-core outputs are tiny (116 x 36); the final mean over (B, W)
    happens on the host.
"""

import numpy as np

B, S, K, D, NNEG = 8, 128, 12, 256, 128
W = S - K          # 116
J = B * S          # 1024
NCORES = 8
MASK_NEG = -60000.0      # fp16-representable "-inf" for unsampled columns

# bf16 blob columns: flatT | cT
BB_FT, BB_CT = 0, 2 * J
BB_END = BB_CT + 2 * W                               # 2280
# fp16 blob columns: nbdl(=ln cnt | -60000) | cnt | ident
H_NB, H_CN, H_ID = 0, J, 2 * J
H_END = H_ID + W                                     # 2164

_CACHE = {}


def _build_program():
    from concourse import bacc, mybir
    import concourse.tile as tile

    f32 = mybir.dt.float32
    bf16 = mybir.dt.bfloat16
    f16 = mybir.dt.float16
    Alu = mybir.AluOpType
    Act = mybir.ActivationFunctionType

    nc = bacc.Bacc(
        "TRN2", target_bir_lowering=False, debug=False, num_devices=NCORES
    )

    fb_d = nc.dram_tensor("fblob", [128, S + K], f32, kind="ExternalInput")
    bb_d = nc.dram_tensor("bblob", [128, BB_END], bf16, kind="ExternalInput")
    hb_d = nc.dram_tensor("hblob", [128, H_END], f16, kind="ExternalInput")
    wp_d = nc.dram_tensor("wpredT", [128, K * 2 * D], bf16, kind="ExternalInput")
    out_d = nc.dram_tensor("out", [W, 3 * K], f32, kind="ExternalOutput")

    with tile.TileContext(nc) as tc:
        with (
            tc.tile_pool(name="consts", bufs=1) as consts,
            tc.tile_pool(name="lcpool", bufs=3) as lcpool,
            tc.tile_pool(name="scr", bufs=2) as scr,
            tc.tile_pool(name="epool", bufs=3) as epool,
            tc.tile_pool(name="upool", bufs=2) as upool,
            tc.tile_pool(name="outs", bufs=1) as outs,
            tc.tile_pool(name="pslc", bufs=2, space="PSUM") as pslc,
            tc.tile_pool(name="pssc", bufs=3, space="PSUM") as pssc,
        ):
            bb = consts.tile([128, BB_END], bf16)
            wpall = consts.tile([128, K * 2 * D], bf16)
            fb = consts.tile([128, S + K], f32)
            hb = consts.tile([128, H_END], f16)
            nc.sync.dma_start(bb[:, BB_CT:BB_END], bb_d[:, BB_CT:BB_END])
            nc.sync.dma_start(wpall[:, 0:512], wp_d[:, 0:512])
            nc.sync.dma_start(bb[:, 0:BB_CT], bb_d[:, 0:BB_CT])
            nc.sync.dma_start(fb[:], fb_d[:])
            nc.sync.dma_start(hb[:, H_ID:H_END], hb_d[:, H_ID:H_END])
            nc.sync.dma_start(hb[:, 0:H_ID], hb_d[:, 0:H_ID])
            nc.sync.dma_start(wpall[:, 512:2048], wp_d[:, 512:2048])
            nc.sync.dma_start(wpall[:, 2048:K * 512], wp_d[:, 2048:K * 512])

            fT_v = bb[:, BB_FT:BB_FT + 2 * J]
            cT_v = bb[:, BB_CT:BB_CT + 2 * W]
            ohb = fb[0:W, 0:S + K]
            nbdl_v = hb[0:W, H_NB:H_NB + J]
            cnt_v = hb[0:W, H_CN:H_CN + J]
            id_v = hb[0:W, H_ID:H_ID + W]

            posS = outs.tile([W, K], f32)
            cntV = outs.tile([W, K], f32)
            negsum = outs.tile([W, K], f32)
            epn = outs.tile([W, K], f32)

            sc_tiles = [None] * K
            ep_tiles = [None] * K

            def emit_front(k):
                # locC_T[k]: (e', ec*W + w) = sum_d WpredT[d, e] * cT[d, w]
                wk = wpall[:, k * 2 * D:(k + 1) * 2 * D]
                lcT_ps = pslc.tile([128, 2 * W], f32, tag="lcT")
                for ec in range(2):
                    for dc in range(2):
                        nc.tensor.matmul(
                            lcT_ps[:, ec * W:(ec + 1) * W],
                            lhsT=wk[:, dc * D + ec * 128: dc * D + (ec + 1) * 128],
                            rhs=cT_v[:, dc * W:(dc + 1) * W],
                            start=(dc == 0),
                            stop=(dc == 1),
                        )
                lcT_bf = lcpool.tile([128, 2 * W], bf16, tag="lcT_bf")
                nc.vector.tensor_copy(lcT_bf[:], lcT_ps[:])

                # scores (bf16): (w, j) = sum_e locC_T[e, w] * flatT[e, j]
                sc_ps = pssc.tile([W, J], f32, tag="sc")
                sc_tiles[k] = sc_ps
                for jc in range(2):
                    for ec in range(2):
                        nc.tensor.matmul(
                            sc_ps[:, jc * 512:(jc + 1) * 512],
                            lhsT=lcT_bf[:, ec * W:(ec + 1) * W],
                            rhs=fT_v[:, ec * J + jc * 512: ec * J + (jc + 1) * 512],
                            start=(ec == 0),
                            stop=False,
                            skip_group_check=True,
                        )

            def emit_pos(k):
                # positive score: exact one-hot extraction at column k+1+w
                # (ohbase[w, c] = 1 iff c == w+13; slicing at 12-k aligns
                # the hot column to j' = w+k+1)
                sc_ps = sc_tiles[k]
                scrP = scr.tile([W, S], f32, tag="scrP")
                nc.vector.scalar_tensor_tensor(
                    out=scrP[:],
                    in0=sc_ps[:, 0:S],
                    scalar=1.0,
                    in1=ohb[:, K - k:K - k + S],
                    op0=Alu.mult,
                    op1=Alu.mult,
                    accum_out=posS[:, k:k + 1],
                )
                if k % 2 == 1:
                    # epn = exp(-pos), batched for the k-1/k pair
                    nc.scalar.activation(
                        out=epn[:, k - 1:k + 1],
                        in_=posS[:, k - 1:k + 1],
                        func=Act.Exp,
                        scale=-1.0,
                    )

            def emit_bias_exp(k):
                # scores += nbdl (ln cnt where sampled, -60000 elsewhere)
                sc_ps = sc_tiles[k]
                for jc in range(2):
                    nc.tensor.matmul(
                        sc_ps[:, jc * 512:(jc + 1) * 512],
                        lhsT=id_v,
                        rhs=nbdl_v[:, jc * 512:(jc + 1) * 512],
                        start=False,
                        stop=(jc == 1),
                        skip_group_check=True,
                    )
                # E = exp(score + nbdl); accum gives negsum = sum cnt*exp(s)
                ep = epool.tile([W, J], bf16, tag="ep")
                ep_tiles[k] = ep
                nc.scalar.activation(
                    out=ep[:],
                    in_=sc_ps[:],
                    func=Act.Exp,
                    accum_out=negsum[:, k:k + 1],
                )

            def emit_count(k):
                # count = sum_j [E * exp(-pos) > cnt]; multiplicity cancels
                # so this is exactly #{sampled j : score_j > pos}
                up = upool.tile([W, J], bf16, tag="up")
                nc.vector.scalar_tensor_tensor(
                    out=up[:],
                    in0=ep_tiles[k][:],
                    scalar=epn[:, k:k + 1],
                    in1=cnt_v[:],
                    op0=Alu.mult,
                    op1=Alu.is_gt,
                    accum_out=cntV[:, k:k + 1],
                )
                ep_tiles[k] = None

            # software pipeline: PE stream stays dense (nbdl of k-1 lands
            # after score of k, far past the posS read of k-1's PSUM)
            for k in range(K):
                emit_front(k)
                emit_pos(k)
                if k > 0:
                    emit_bias_exp(k - 1)
                if k > 1:
                    emit_count(k - 2)
            emit_bias_exp(K - 1)
            emit_count(K - 2)
            emit_count(K - 1)

            nc.sync.dma_start(out_d[:, 0:K], negsum[:])
            nc.sync.dma_start(out_d[:, K:2 * K], posS[:])
            nc.sync.dma_start(out_d[:, 2 * K:3 * K], cntV[:])

    nc.compile()
    return nc


def _host_prep(cFeature, encodedData, Wpred, batchIdx, seqIdx):
    import ml_dtypes

    bf = ml_dtypes.bfloat16
    cF = np.ascontiguousarray(np.asarray(cFeature, dtype=np.float32))
    eD = np.ascontiguousarray(np.asarray(encodedData, dtype=np.float32))
    Wp = np.ascontiguousarray(np.asarray(Wpred, dtype=np.float32))
    bI = np.asarray(batchIdx).astype(np.int64)
    sI = np.asarray(seqIdx).astype(np.int64)

    flat = eD.reshape(J, D)
    idx = np.arange(NNEG * W * B, dtype=np.int64)
    ext = ((sI + idx % W) % S + bI * S).reshape(B, NNEG, W)

    wt = Wp.transpose(0, 2, 1)  # (K, d, e)
    wp_host = np.concatenate(
        [np.concatenate([wt[k, :128, :], wt[k, 128:, :]], axis=1) for k in range(K)],
        axis=1,
    ).astype(bf)  # (128, K*2D)
    wp_host = np.ascontiguousarray(wp_host)

    fblob = np.zeros((128, S + K), np.float32)
    fblob[np.arange(W), np.arange(W) + K + 1] = 1.0

    rows = np.tile(np.arange(W), NNEG)
    in_maps = []
    cnts_orig = []
    for b in range(B):
        perm = np.r_[b * S:(b + 1) * S, 0:b * S, (b + 1) * S:J]
        inv = np.empty(J, np.int64)
        inv[perm] = np.arange(J)

        fT = flat[perm].T  # (D, J) fp32
        cT = cF[b, :W].T * np.float32(1.0 / 256.0)  # exact power-of-2 scale

        cnt = np.zeros((W, J), np.float32)
        np.add.at(cnt, (rows, inv[ext[b].ravel()]), 1.0)
        cnt_o = np.zeros((W, J), np.float32)
        np.add.at(cnt_o, (rows, ext[b].ravel()), 1.0)
        cnts_orig.append(cnt_o)
        nz = cnt > 0

        bblob = np.zeros((128, BB_END), bf)
        bblob[:, BB_FT:BB_FT + J] = fT[:128].astype(bf)
        bblob[:, BB_FT + J:BB_FT + 2 * J] = fT[128:].astype(bf)
        bblob[:, BB_CT:BB_CT + W] = cT[:128].astype(bf)
        bblob[:, BB_CT + W:BB_CT + 2 * W] = cT[128:].astype(bf)

        hblob = np.zeros((128, H_END), np.float16)
        nbdl = np.full((W, J), MASK_NEG, np.float32)
        nbdl[nz] = np.log(cnt[nz])
        hblob[:W, H_NB:H_NB + J] = nbdl.astype(np.float16)
        hblob[:W, H_CN:H_CN + J] = cnt.astype(np.float16)
        hblob[:W, H_ID:H_ID + W] = np.eye(W, dtype=np.float16)

        in_maps.append({
            "fblob": fblob,
            "bblob": np.ascontiguousarray(bblob),
            "hblob": np.ascontiguousarray(hblob),
            "wpredT": wp_host,
        })
    return in_maps, cnts_orig, flat, cF, Wp


def _host_fix_acc(acc01, counts, cnts_orig, flat, cF, Wp):
    """Re-resolve near-boundary accuracy bits (count <= 1) in float64."""
    flat64 = flat.astype(np.float64)
    for b in range(B):
        flag = counts[b] <= 1.0                  # (W, K)
        for w, k in zip(*np.nonzero(flag)):
            lc = (cF[b, w].astype(np.float64) / 256.0) @ Wp[k].astype(np.float64).T
            sc = flat64 @ lc                     # (J,)
            mn = sc[cnts_orig[b][w] > 0].max()
            p = sc[b * S + k + 1 + w]
            acc01[b, w, k] = 1.0 if p >= mn else 0.0
    return acc01


def kernel(cFeature, encodedData, Wpred, batchIdx, seqIdx, _trace=False):
    from concourse.bass_utils import run_bass_kernel_spmd

    in_maps, cnts_orig, flat, cF, Wp = _host_prep(
        cFeature, encodedData, Wpred, batchIdx, seqIdx
    )

    if "nc" not in _CACHE:
        _CACHE["nc"] = _build_program()
    nc = _CACHE["nc"]

    kw = {}
    if _trace:
        kw = {"trace": True}
    res = run_bass_kernel_spmd(nc, in_maps, core_ids=list(range(NCORES)), **kw)
    _CACHE["last_results"] = res

    outs = np.stack([res.results[b]["out"] for b in range(B)])  # (B, W, 3K)
    negsum = outs[:, :, :K].astype(np.float64)
    posS = outs[:, :, K:2 * K]
    counts = outs[:, :, 2 * K:3 * K]
    p64 = posS.astype(np.float64)
    lossc = np.log(negsum + np.exp(p64)) - p64

    acc01 = (counts == 0).astype(np.float32)
    acc01 = _host_fix_acc(acc01, counts, cnts_orig, flat, cF, Wp)

    losses = lossc.sum(axis=(0, 1), dtype=np.float64) / (B * W)
    accs = acc01.sum(axis=(0, 1), dtype=np.float64) / (B * W)
    return (
        losses.astype(np.float32)[None, :],
        accs.astype(np.float32)[None, :],
    )
